# revision 57
# baseline (speedup 1.0000x reference)
"""Multi-head attention (B=2, S=2048, D=1024, H=16) on 8 Trainium2 NeuronCores.

Sharding: core c handles batch b = c//4 and head group g = c%4 (4 heads = 2
head-pairs, 256 model dims).  Each core computes q/k/v projections for its
heads, attention, and a partial output projection (row-parallel over its 256
head dims); the host sums the 4 partials per batch and adds the bias.

Layouts put the tensor-engine contraction dim on SBUF partitions everywhere:
  xT [d, s], qT/kT [e(128 = head pair), s] fp32r
  scores sc [ks, qs] fp32r -> exp on ACT -> p [ks, qs] bf16 in SBUF
  v [ks, e+ones] bf16 per (ks-block, head)
  PV is computed FLIPPED: out[qs(128), e(65)] = p_chunk^T @ v_blk, which uses
  all 128 output partitions at full bf16 rate (cost model charges per output
  row) instead of the 65-partition [e, qs] orientation, and row 64 (the ones
  column) accumulates the softmax denominator.  Normalization is then a
  per-partition reciprocal + scale (no partition broadcast, no DRAM
  round-trips), written bf16 and transposed back to the [e, qs] head-pair
  layout with a tensor-engine transpose against a bf16 identity.

PSUM allows only ONE matmul accumulation group per 2KB bank (and GPSIMD may
not touch PSUM at all), so per attention group the 16 (head, qs-block)
accumulations run as TWO passes of 8-matmul chains through 2 rotating banks
(first-half partials parked in SBUF, fused back with one DVE add), hooked
under the adjacent group's score/exp stream at matched priority.  Deferred
q/k/v projections and the overlapped output projection stream through two
more alternating banks as low-priority hooks; scores keep 4 banks.  The last
group's second pass is the tail: four carriers plus the freed score banks
keep its normalize/transpose/project chains unserialized.

Matmuls: fp32r for x/w/q/k/scores (accuracy), bf16 for p/v/attn/wo (~0.2%
element error, ~3.5e-3 rel err overall).  Output partials are bf16, summed
with the bias on the host.
"""

import os
import sys

import numpy as np

for _p in ("/opt/trn_rl_repo", "/root/.axon_site/_ro/trn_rl_repo"):
    if os.path.isdir(_p) and _p not in sys.path:
        sys.path.insert(0, _p)

import bass_rust
import concourse.bass as bass
import concourse.mybir as mybir
import concourse.tile as tile
from concourse.bass_utils import run_bass_kernel_spmd
from concourse.vector_clock import ScopedClock, VectorClock
from contextlib import ExitStack

F32 = mybir.dt.float32
F32R = mybir.dt.float32r
BF16 = mybir.dt.bfloat16
EXP = mybir.ActivationFunctionType.Exp

B = 2
S = 2048
D = 1024
H = 16
HD = 64
NCORES = 8
GROUPS = 4          # head groups (cores per batch)
HG = H // GROUPS    # heads per core = 4
E = HG * HD         # head dims per core = 256
KT = D // 128       # contraction tiles over model dim = 8
SB = S // 128       # 128-wide s blocks = 16
QB = S // 512       # 512-wide s blocks = 4

_carrier_counter = [0]


def _split_multi_waits(ordered):
    """This walrus build allows one sync wait per instruction; Tile's wait
    assignment can attach several.  Hoist extras onto same-engine InstNoOp
    carriers placed immediately before the instruction."""
    for bb_name, insts in ordered.items():
        new_list = []
        for inst in insts:
            si = inst.sync_info
            waits = list(si.on_wait) if si is not None else []
            if len(waits) > 1:
                for w in waits[:-1]:
                    _carrier_counter[0] += 1
                    carrier = mybir.InstNoOp(
                        name=f"I-waitc-{_carrier_counter[0]}", ins=[], outs=[]
                    )
                    carrier.engine = inst.engine
                    carrier.sync_info = bass_rust.SyncInfo(on_wait=[w], on_update=[])
                    new_list.append(carrier)
                inst.sync_info = bass_rust.SyncInfo(
                    on_wait=[waits[-1]],
                    on_update=list(si.on_update) if si is not None else [],
                )
            new_list.append(inst)
        ordered[bb_name] = new_list


class _TileContext(tile.TileContext):
    """TileContext adapted to the one-sync-wait-per-instruction walrus."""

    def _lower_ordered_insts(self, ordered):
        _split_multi_waits(ordered)
        return super()._lower_ordered_insts(ordered)

    def _drain_and_barrier(self, tick_clock, wait_clock):
        gc = tick_clock.global_clock
        for proc in range(len(gc)):
            if gc[proc] <= 0:
                continue
            cur = VectorClock([0 if i == proc else gc[i] for i in range(len(gc))])
            nop = self.nc.sync.nop()
            wait_clock.add_sem_waits(
                nop.ins, ScopedClock({None: gc}), ScopedClock({None: cur})
            )
        drain_inst = self.nc.sync.drain()
        wait_clock.add_sem_waits(
            drain_inst.ins, ScopedClock({None: gc}), ScopedClock({None: gc.copy()})
        )
        self.nc.all_engine_barrier()
        assert self.sems is not None
        popped = self.nc._tile_sem_poison_stack.pop()
        assert popped is self._sem_poison
        self.nc.clear_and_free_semaphores(list(self.sems.allocated().values()))
        self.nc.all_engine_barrier()


def build_nc(reps=1):
    nc = bass.Bass()
    xT = nc.declare_dram_parameter("xT", [D, S], F32R, isOutput=False)
    wqT = nc.declare_dram_parameter("wqT", [D, E], F32R, isOutput=False)
    wkT = nc.declare_dram_parameter("wkT", [D, E], F32R, isOutput=False)
    wvT = nc.declare_dram_parameter("wvT", [D, E], F32R, isOutput=False)
    woT = nc.declare_dram_parameter("woT", [E, D], BF16, isOutput=False)
    ident_d = nc.declare_dram_parameter("ident", [128, 128], BF16, isOutput=False)
    out = nc.declare_dram_parameter("out_partial", [S, D], BF16, isOutput=True)

    with _TileContext(nc) as tc, ExitStack() as outer:
      for _rep in range(reps):
        ctx = outer.enter_context(ExitStack())
        # ---- persistent tiles ----
        act_pool = ctx.enter_context(tc.tile_pool(name="acts", bufs=1))
        qT_sb = [act_pool.tile([128, S], F32R, tag=f"qT{m}", name=f"qT{m}") for m in range(2)]
        kT_sb = [act_pool.tile([128, S], F32R, tag=f"kT{m}", name=f"kT{m}") for m in range(2)]
        v_sb = act_pool.tile([128, SB, HG, HD + 1], BF16, tag="v")
        wo_sb = act_pool.tile([128, 2, D], BF16, tag="wo")
        ident = act_pool.tile([128, 128], BF16, tag="id")
        attn_pair = [act_pool.tile([128, S], BF16, tag=f"ap{m}", name=f"ap{m}") for m in range(2)]

        # deferred-projection psum bank (right stack, below xqk so xqk can
        # close first); also reused for the overlapped output projection
        cdef = ExitStack()
        dpool = cdef.enter_context(
            tc.tile_pool(name="dp", bufs=1, space="PSUM", side="right")
        )

        # ---- x + weights (freed after the last deferred projection) ----
        c1x = ctx.enter_context(ExitStack())
        xqk_pool = c1x.enter_context(tc.tile_pool(name="xqk", bufs=1, side="right"))
        x_sb = xqk_pool.tile([128, KT, S], F32R, tag="x")
        wq_sb = xqk_pool.tile([128, KT, E], F32R, tag="wq")
        wk_sb = xqk_pool.tile([128, KT, E], F32R, tag="wk")
        cwv = ExitStack()
        wv_pool = cwv.enter_context(
            tc.tile_pool(name="wvp", bufs=1, side="right")
        )
        wv_sb = wv_pool.tile([128, KT, E], F32R, tag="wv")

        nc.vector.memset(v_sb[:, :, :, HD], 1.0)

        # DMA order (the model serializes the shared DMA device in issue
        # order): wq, wk, wv (so v projections can run in the pre-attention
        # PE window), then all of x, wo, ident.
        def dma_x(nb):
            for k in range(KT):
                eng = nc.sync if k % 2 == 0 else nc.gpsimd
                eng.dma_start(
                    x_sb[:, k, nb * 512:(nb + 1) * 512],
                    xT[k * 128:(k + 1) * 128, nb * 512:(nb + 1) * 512],
                )

        for k in range(KT):
            eng = nc.sync if k % 2 == 0 else nc.gpsimd
            eng.dma_start(wq_sb[:, k, :], wqT[k * 128:(k + 1) * 128, :])
        dma_x(0)
        for k in range(KT):
            eng = nc.sync if k % 2 == 0 else nc.gpsimd
            eng.dma_start(wk_sb[:, k, :], wkT[k * 128:(k + 1) * 128, :])
        for k in range(KT):
            eng = nc.sync if k % 2 == 0 else nc.gpsimd
            eng.dma_start(wv_sb[:, k, :], wvT[k * 128:(k + 1) * 128, :])
        for nb in range(1, 4):
            dma_x(nb)
        for m in range(2):
            eng = nc.sync if m == 0 else nc.gpsimd
            eng.dma_start(wo_sb[:, m, :], woT[m * 128:(m + 1) * 128, :])
        nc.sync.dma_start(ident[:], ident_d[:, :])

        def proj_qk(dst, w_sb, mcol, nb, ps):
            """dst[:, nb*512:+512] = (w column block mcol)^T x, via psum ps."""
            for k in range(KT):
                nc.tensor.matmul(
                    ps[:],
                    w_sb[:, k, mcol * 128:(mcol + 1) * 128],
                    x_sb[:, k, nb * 512:(nb + 1) * 512],
                    start=(k == 0),
                    stop=(k == KT - 1),
                )

        copy_flip = [0]
        copy_mode = ["startup"]  # pre-attention: ACT is idle

        def drain_copy(dst_ap, src_ap):
            # GPSIMD cannot access PSUM, so psum-draining copies go to DVE,
            # plus ACT while it is still idle (before the first exp)
            copy_flip[0] += 1
            if copy_mode[0] == "startup" and copy_flip[0] % 2 == 1:
                nc.scalar.copy(dst_ap, src_ap)
            else:
                nc.vector.tensor_copy(dst_ap, src_ap)

        # ---- phase 1a: qT0 half 0 + kT0 (gates the first attention group) --
        with ExitStack() as c1a:
            pp = c1a.enter_context(tc.tile_pool(name="pp", bufs=3, space="PSUM"))
            for nb in range(2):
                ps = pp.tile([128, 512], F32, tag="pp")
                proj_qk(qT_sb[0], wq_sb, 0, nb, ps)
                drain_copy(qT_sb[0][:, nb * 512:(nb + 1) * 512], ps[:])
            for nb in range(2):
                ps = pp.tile([128, 512], F32, tag="pp")
                proj_qk(kT_sb[0], wk_sb, 0, nb, ps)
                drain_copy(kT_sb[0][:, nb * 512:(nb + 1) * 512], ps[:])

        # ---- deferred projections.  PSUM allows only ONE accumulation
        # group per 2KB bank, so each unit owns a full bank; two tags
        # alternate banks so consecutive units overlap without parking in
        # PE's wait queue.  Units stream as low-priority hooks.
        dflip = [0]

        def dnext():
            dflip[0] += 1
            return dpool.tile(
                [128, 512], F32, tag="dA" if dflip[0] % 2 else "dB",
                name=f"dt{dflip[0]}",
            )

        def def_v(sb):
            """v projection for one 128-row s-block."""
            ps = dnext()[:, 0:256]
            for k in range(KT):
                nc.tensor.matmul(
                    ps,
                    x_sb[:, k, sb * 128:(sb + 1) * 128],
                    wv_sb[:, k, :],
                    start=(k == 0),
                    stop=(k == KT - 1),
                )
            drain_copy(
                v_sb[:, sb, :, 0:HD],
                ps.rearrange("p (h e) -> p h e", h=HG),
            )

        def def_qk(dst, w_sb, mcol, nb):
            """one 512-wide q/k output block."""
            ps = dnext()[:]
            proj_qk(dst, w_sb, mcol, nb, ps)
            drain_copy(dst[:, nb * 512:(nb + 1) * 512], ps)

        # v s-blocks 0-7 fit the pre-attention PE window (wv + x halves 0-1)
        for sb in range(8):
            def_v(sb)

        def qk_unit(dst, w_sb, mcol, nb):
            return lambda: def_qk(dst, w_sb, mcol, nb)

        # group order is (m0,qh0),(m0,qh1),(m1,qh0),(m1,qh1): group 1 reuses
        # kT0, so every deferred projection has at least a full group of
        # deadline slack.
        def_units = {
            0: (
                [qk_unit(kT_sb[0], wk_sb, 0, 2),
                 qk_unit(qT_sb[0], wq_sb, 0, 2)]
                + [(lambda s: (lambda: def_v(s)))(s) for s in (8, 9, 10)]
                + [qk_unit(kT_sb[0], wk_sb, 0, 3),
                   qk_unit(qT_sb[0], wq_sb, 0, 3)]
                + [(lambda s: (lambda: def_v(s)))(s) for s in (11, 12, 13, 14, 15)]
            ),
            1: [
                qk_unit(kT_sb[1], wk_sb, 1, 0),
                qk_unit(kT_sb[1], wk_sb, 1, 1),
                qk_unit(qT_sb[1], wq_sb, 1, 0),
                qk_unit(qT_sb[1], wq_sb, 1, 1),
                qk_unit(kT_sb[1], wk_sb, 1, 2),
                qk_unit(kT_sb[1], wk_sb, 1, 3),
            ],
            2: [
                qk_unit(qT_sb[1], wq_sb, 1, 2),
                qk_unit(qT_sb[1], wq_sb, 1, 3),
            ],
        }

        # ---- phase 2: attention groups ----
        # Phase A per group: scores + exp, retaining all 32 p tiles.
        # Phase B (hooked under the NEXT group's phase A): per (head,
        # qs-block) a 16-matmul accumulation chain through a single psum
        # bank (ones column gives the denominator in row 64), then
        # reciprocal + scale (bf16) + tensor-engine transpose back to the
        # [e, qs] head-pair tile.
        ost_pool = ctx.enter_context(tc.tile_pool(name="ost", bufs=4))
        grp = ctx.enter_context(ExitStack())
        sc_pool = grp.enter_context(tc.tile_pool(name="sc", bufs=1, space="PSUM"))
        pv_pool = grp.enter_context(tc.tile_pool(name="pv", bufs=1, space="PSUM"))
        p_pool = grp.enter_context(tc.tile_pool(name="pexp", bufs=27))
        st_pool = grp.enter_context(tc.tile_pool(name="stg", bufs=6))
        rd_pool = grp.enter_context(tc.tile_pool(name="rd", bufs=8))
        cast_flip = [0]

        def tail_cast(dst, src, use_act):
            cast_flip[0] += 1
            if use_act and cast_flip[0] % 2 == 0:
                nc.scalar.copy(dst, src)
            else:
                nc.vector.tensor_copy(dst, src)

        def tail_proj(sb, use_act=True):
            """output projection of one s-block via the deferred banks
            (DVE casts while ACT still runs exps; gpsimd cannot read
            psum)."""
            for nb in range(2):
                ps = dnext()[:]
                for mm in range(2):
                    nc.tensor.matmul(
                        ps,
                        attn_pair[mm][:, sb * 128:(sb + 1) * 128],
                        wo_sb[:, mm, nb * 512:(nb + 1) * 512],
                        start=(mm == 0),
                        stop=(mm == 1),
                    )
                st_o = ost_pool.tile([128, 512], BF16, tag="ost")
                tail_cast(st_o[:], ps, use_act)
                nc.sync.dma_start(
                    out[sb * 128:(sb + 1) * 128, nb * 512:(nb + 1) * 512],
                    st_o[:],
                )

        part_pool = grp.enter_context(tc.tile_pool(name="part", bufs=27))
        car_n = [0]

        def pv_car(tag):
            def alloc():
                car_n[0] += 1
                return pv_pool.tile(
                    [128, HD + 1], F32, tag=tag, name=f"car{car_n[0]}"
                )[:]
            return alloc

        mid_cars = [pv_car("pvA"), pv_car("pvB")]

        def chain(car, pts, m, r, qsb, k0):
            """8-matmul accumulation over ksb k0..k0+7 into a psum carrier."""
            for kk in range(8):
                ksb = k0 + kk
                nc.tensor.matmul(
                    car,
                    pts[r][ksb][:, qsb * 128:(qsb + 1) * 128],
                    v_sb[:, ksb, 2 * m + r, :],
                    start=(kk == 0),
                    stop=(kk == 7),
                )

        def make_half1(m, pts, parts, carriers):
            """First-half chains (ksb 0-7), partials parked in SBUF; hooked
            under the same group's second-half scores."""
            items = []

            def one(qsb, r):
                def run():
                    car = carriers[(qsb * 2 + r) % len(carriers)]()
                    chain(car, pts, m, r, qsb, 0)
                    pt = part_pool.tile([128, HD + 1], F32, tag="pt",
                                        name=f"pt{qsb}_{r}")
                    nc.vector.tensor_copy(pt[:], car)
                    parts[qsb * 2 + r] = pt
                return run

            for qsb in range(8):
                for r in range(2):
                    items.append(one(qsb, r))
            return items

        def make_half2(gi, m, qh, pts, parts, carriers, trt, n_trs, tail,
                      proj_base):
            """Second-half chains + partial add + normalize + transpose
            (+ output projection); hooked under the next group, or emitted
            directly as the tail for the last group."""
            items = []
            sts = {}

            def slot(qsb, r):
                def run():
                    car = carriers[(qsb * 2 + r) % len(carriers)]()
                    chain(car, pts, m, r, qsb, 8)
                    pt = parts[qsb * 2 + r]
                    nc.vector.tensor_add(pt[:], pt[:], car)
                    rden = rd_pool.tile([128, 1], F32, tag="rd")
                    nc.vector.reciprocal(rden[:], pt[:, 64:65])
                    if qsb * 2 + r in sts:
                        st = sts[qsb * 2 + r]
                    else:
                        st = st_pool.tile([128, 128], BF16, tag="st")
                        sts[qsb * 2] = st
                        sts[qsb * 2 + 1] = st
                    if tail and r == 0:
                        nc.scalar.mul(
                            st[:, r * 64:(r + 1) * 64], pt[:, 0:HD], rden[:]
                        )
                    else:
                        nc.vector.tensor_scalar_mul(
                            st[:, r * 64:(r + 1) * 64], pt[:, 0:HD], rden[:]
                        )
                return run

            def tr_item(qsb):
                def run():
                    s0 = (qsb % n_trs) * 64
                    tr_ap = trt()[:, s0:s0 + 64].bitcast(BF16)
                    nc.tensor.matmul(
                        tr_ap, sts[qsb * 2][:], ident[:], is_transpose=True
                    )
                    attn_dst = attn_pair[m][:, qh * 1024 + qsb * 128:
                                            qh * 1024 + (qsb + 1) * 128]
                    nc.vector.tensor_copy(attn_dst, tr_ap)
                    if proj_base is not None:
                        tail_proj(proj_base + qsb, use_act=tail)
                return run

            items.append(slot(0, 0))
            items.append(slot(0, 1))
            for qsb in range(1, 8):
                items.append(slot(qsb, 0))
                items.append(tr_item(qsb - 1))
                items.append(slot(qsb, 1))
            items.append(tr_item(7))
            return items

        def lowprio(fn):
            orig_prio = tc.cur_priority
            tc.cur_priority = orig_prio + 500000
            try:
                fn()
            finally:
                tc.cur_priority = orig_prio

        group_list = [(0, 0), (0, 1), (1, 0), (1, 1)]  # (m, qh)
        copy_mode[0] = "era"
        prevB = []
        for gi, (m, qh) in enumerate(group_list):
            # hook streams: previous group's second half (normal priority —
            # it recycles the p tiles the exps need) and deferred
            # projections (low priority)
            ghA = list(prevB)
            ghD = list(def_units.get(gi, []))
            pts = [[None] * SB, [None] * SB]
            parts = [None] * 16
            ghB = []   # own first-half chains, emitted under ksb 8-15
            for ksb in range(SB):
                for r in range(2):
                    sc = sc_pool.tile([128, 1024], F32, tag=f"sc{r}")
                    for qq in range(2):
                        nc.tensor.matmul(
                            sc[:, qq * 512:(qq + 1) * 512],
                            kT_sb[m][64 * r:64 * r + 64,
                                     ksb * 128:(ksb + 1) * 128],
                            qT_sb[m][64 * r:64 * r + 64,
                                     qh * 1024 + qq * 512:
                                     qh * 1024 + (qq + 1) * 512],
                            start=True,
                            stop=True,
                        )
                    p = p_pool.tile([128, 1024], BF16, tag="p", name=f"p{r}")
                    nc.scalar.activation(p[:], sc[:], EXP)
                    pts[r][ksb] = p
                if ksb == 7:
                    ghB = make_half1(m, pts, parts, mid_cars)
                for _ in range(3):
                    if ghA:
                        ghA.pop(0)()
                for _ in range(3):
                    if ghB:
                        ghB.pop(0)()
                if ghD:
                    lowprio(ghD.pop(0))
            for it in ghA + ghB:
                it()
            for it in ghD:
                lowprio(it)
            if gi == 0:
                cwv.close()    # wv SBUF free (v fully projected)
            elif gi == 2:
                c1x.close()    # x / wq / wk SBUF free
            if gi < 3:
                trm_n = [0]

                def trm():
                    trm_n[0] += 1
                    return pv_pool.tile([128, HD + 1], F32, tag="pvA",
                                        name=f"trm{gi}_{trm_n[0]}")
                prevB = make_half2(
                    gi, m, qh, pts, parts, mid_cars,
                    trt=trm, n_trs=1, tail=False,
                    proj_base=(0 if gi == 2 else None),
                )
            else:
                # last group: the second half is the tail.  Four carriers
                # (pv banks + freed sc0 banks), transposes in freed sc1.
                scc = sc_pool.tile([128, 1024], F32, tag="sc0", name="otc")
                tr3 = sc_pool.tile([128, 1024], F32, tag="sc1", name="tr3")
                tailB = make_half2(
                    gi, m, qh, pts, parts,
                    carriers=[
                        pv_car("pvA"), pv_car("pvB"),
                        lambda: scc[:, 0:HD + 1],
                        lambda: scc[:, 512:512 + HD + 1],
                    ],
                    trt=lambda: tr3, n_trs=8, tail=True,
                    proj_base=8,
                )
                for it in tailB:
                    it()
        grp.close()
        cdef.close()
        ctx.close()
    return nc


_NC_CACHE = None


def _get_nc():
    global _NC_CACHE
    if _NC_CACHE is None:
        _NC_CACHE = build_nc()
    return _NC_CACHE


_EXEC_CACHE = None


def _get_executor():
    """Build + jit the SPMD executable once; reuse across kernel() calls.

    Mirrors concourse.bass2jax.run_bass_via_pjrt, which re-jits on every
    call (full retrace + executable reload); caching shaves seconds/call."""
    global _EXEC_CACHE
    if _EXEC_CACHE is not None:
        return _EXEC_CACHE
    import jax
    from jax.sharding import Mesh, PartitionSpec
    from jax.experimental.shard_map import shard_map
    from concourse import bass2jax as b2j

    nc = _get_nc()
    b2j.install_neuronx_cc_hook()
    assert nc.dbg_addr is None
    partition_name = (
        nc.partition_id_tensor.name if nc.partition_id_tensor is not None else None
    )

    in_names, out_names, out_avals = [], [], []
    for alloc in nc.m.functions[0].allocations:
        if not isinstance(alloc, mybir.MemoryLocationSet):
            continue
        name = alloc.memorylocations[0].name
        if alloc.kind == "ExternalInput":
            if name != partition_name:
                in_names.append(name)
        elif alloc.kind == "ExternalOutput":
            out_names.append(name)
            out_avals.append(
                jax.core.ShapedArray(
                    tuple(alloc.tensor_shape), mybir.dt.np(alloc.dtype)
                )
            )
    n_params = len(in_names)
    n_outs = len(out_avals)
    all_names = in_names + out_names
    if partition_name is not None:
        all_names = all_names + [partition_name]

    def _body(*args):
        operands = list(args)
        if partition_name is not None:
            operands.append(b2j.partition_id_tensor())
        outs = b2j._bass_exec_p.bind(
            *operands,
            out_avals=tuple(out_avals),
            in_names=tuple(all_names),
            out_names=tuple(out_names),
            lowering_input_output_aliases=(),
            sim_require_finite=True,
            sim_require_nnan=True,
            nc=nc,
        )
        return tuple(outs)

    devices = jax.devices()[:NCORES]
    mesh = Mesh(np.asarray(devices), ("core",))
    donate = tuple(range(n_params, n_params + n_outs))
    sharded = jax.jit(
        shard_map(
            _body,
            mesh=mesh,
            in_specs=(PartitionSpec("core"),) * (n_params + n_outs),
            out_specs=(PartitionSpec("core"),) * n_outs,
            check_rep=False,
        ),
        donate_argnums=donate,
        keep_unused=True,
    )
    import jax.numpy as jnp

    zero_shardings = [
        jax.sharding.NamedSharding(mesh, PartitionSpec("core"))
    ] * n_outs

    @jax.jit
    def _make_zeros():
        return tuple(
            jax.lax.with_sharding_constraint(
                jnp.zeros((NCORES * a.shape[0], *a.shape[1:]), a.dtype), sh
            )
            for a, sh in zip(out_avals, zero_shardings)
        )

    _EXEC_CACHE = {
        "sharded": sharded,
        "make_zeros": _make_zeros,
        "in_names": in_names,
        "out_names": out_names,
        "out_avals": out_avals,
    }
    return _EXEC_CACHE


def _run_spmd(in_maps):
    ex = _get_executor()
    concat_in = [
        np.concatenate([np.asarray(m[name]) for m in in_maps], axis=0)
        for name in ex["in_names"]
    ]
    concat_zeros = ex["make_zeros"]()
    out_arrs = ex["sharded"](*concat_in, *concat_zeros)
    results = []
    for c in range(NCORES):
        results.append({
            name: np.asarray(out_arrs[i]).reshape(
                NCORES, *ex["out_avals"][i].shape
            )[c]
            for i, name in enumerate(ex["out_names"])
        })
    return results


def _shard_inputs(x, Wq, Wk, Wv, Wo):
    import ml_dtypes

    bf16 = ml_dtypes.bfloat16
    scale = np.float32(1.0 / np.sqrt(HD))
    ident = np.eye(128, dtype=bf16)
    in_maps = []
    xT_b = [np.ascontiguousarray(x[b].T) for b in range(B)]
    for c in range(NCORES):
        b, g = divmod(c, GROUPS)
        sl = slice(g * E, (g + 1) * E)
        in_maps.append({
            "ident": ident,
            "xT": xT_b[b],
            "wqT": np.ascontiguousarray(Wq[sl, :].T * scale),
            "wkT": np.ascontiguousarray(Wk[sl, :].T),
            "wvT": np.ascontiguousarray(Wv[sl, :].T),
            "woT": np.ascontiguousarray(Wo[:, sl].T).astype(bf16),
        })
    return in_maps


_FAST_PATH_OK = True


def kernel(x, Wq, Wk, Wv, Wo, bo):
    global _FAST_PATH_OK
    x = np.asarray(x, dtype=np.float32)
    in_maps = _shard_inputs(
        x,
        np.asarray(Wq, dtype=np.float32),
        np.asarray(Wk, dtype=np.float32),
        np.asarray(Wv, dtype=np.float32),
        np.asarray(Wo, dtype=np.float32),
    )
    results = None
    if _FAST_PATH_OK:
        try:
            results = _run_spmd(in_maps)
        except Exception:
            _FAST_PATH_OK = False
    if results is None:
        # portable fallback: stock SPMD runner (handles native-device
        # environments and anything the cached-PJRT fast path can't)
        results = run_bass_kernel_spmd(
            _get_nc(), in_maps, list(range(NCORES))
        ).results
    bo = np.asarray(bo, dtype=np.float32)
    out = np.empty((B, S, D), dtype=np.float32)
    for b in range(B):
        acc = np.zeros((S, D), dtype=np.float64)
        for g in range(GROUPS):
            acc += np.asarray(results[b * GROUPS + g]["out_partial"],
                              dtype=np.float64)
        out[b] = (acc + bo.astype(np.float64)).astype(np.float32)
    return out


# revision 61
# speedup vs baseline: 1.0090x; 1.0090x over previous
"""Multi-head attention (B=2, S=2048, D=1024, H=16) on 8 Trainium2 NeuronCores.

Sharding: core c handles batch b = c//4 and head group g = c%4 (4 heads = 2
head-pairs, 256 model dims).  Each core computes q/k/v projections for its
heads, attention, and a partial output projection (row-parallel over its 256
head dims); the host sums the 4 partials per batch and adds the bias.

Layouts put the tensor-engine contraction dim on SBUF partitions everywhere:
  xT [d, s], qT/kT [e(128 = head pair), s] fp32r
  scores sc [ks, qs] fp32r -> exp on ACT -> p [ks, qs] bf16 in SBUF
  v [ks, e+ones] bf16 per (ks-block, head)
  PV is computed FLIPPED: out[qs(128), e(65)] = p_chunk^T @ v_blk, which uses
  all 128 output partitions at full bf16 rate (cost model charges per output
  row) instead of the 65-partition [e, qs] orientation, and row 64 (the ones
  column) accumulates the softmax denominator.  Normalization is then a
  per-partition reciprocal + scale (no partition broadcast, no DRAM
  round-trips), written bf16 and transposed back to the [e, qs] head-pair
  layout with a tensor-engine transpose against a bf16 identity.

PSUM allows only ONE matmul accumulation group per 2KB bank (and GPSIMD may
not touch PSUM at all), so per attention group the 16 (head, qs-block)
accumulations run as TWO passes of 8-matmul chains through 2 rotating banks
(first-half partials parked in SBUF, fused back with one DVE add), hooked
under the adjacent group's score/exp stream at matched priority.  Deferred
q/k/v projections and the overlapped output projection stream through two
more alternating banks as low-priority hooks; scores keep 4 banks.  The last
group's second pass is the tail: four carriers plus the freed score banks
keep its normalize/transpose/project chains unserialized.

Matmuls: fp32r for x/w/q/k/scores (accuracy), bf16 for p/v/attn/wo (~0.2%
element error, ~3.5e-3 rel err overall).  Output partials are bf16, summed
with the bias on the host.
"""

import os
import sys

import numpy as np

for _p in ("/opt/trn_rl_repo", "/root/.axon_site/_ro/trn_rl_repo"):
    if os.path.isdir(_p) and _p not in sys.path:
        sys.path.insert(0, _p)

import bass_rust
import concourse.bass as bass
import concourse.mybir as mybir
import concourse.tile as tile
from concourse.bass_utils import run_bass_kernel_spmd
from concourse.vector_clock import ScopedClock, VectorClock
from contextlib import ExitStack

F32 = mybir.dt.float32
F32R = mybir.dt.float32r
BF16 = mybir.dt.bfloat16
EXP = mybir.ActivationFunctionType.Exp

B = 2
S = 2048
D = 1024
H = 16
HD = 64
NCORES = 8
GROUPS = 4          # head groups (cores per batch)
HG = H // GROUPS    # heads per core = 4
E = HG * HD         # head dims per core = 256
KT = D // 128       # contraction tiles over model dim = 8
SB = S // 128       # 128-wide s blocks = 16
QB = S // 512       # 512-wide s blocks = 4

_carrier_counter = [0]


def _split_multi_waits(ordered):
    """This walrus build allows one sync wait per instruction; Tile's wait
    assignment can attach several.  Hoist extras onto same-engine InstNoOp
    carriers placed immediately before the instruction."""
    for bb_name, insts in ordered.items():
        new_list = []
        for inst in insts:
            si = inst.sync_info
            waits = list(si.on_wait) if si is not None else []
            if len(waits) > 1:
                for w in waits[:-1]:
                    _carrier_counter[0] += 1
                    carrier = mybir.InstNoOp(
                        name=f"I-waitc-{_carrier_counter[0]}", ins=[], outs=[]
                    )
                    carrier.engine = inst.engine
                    carrier.sync_info = bass_rust.SyncInfo(on_wait=[w], on_update=[])
                    new_list.append(carrier)
                inst.sync_info = bass_rust.SyncInfo(
                    on_wait=[waits[-1]],
                    on_update=list(si.on_update) if si is not None else [],
                )
            new_list.append(inst)
        ordered[bb_name] = new_list


class _TileContext(tile.TileContext):
    """TileContext adapted to the one-sync-wait-per-instruction walrus."""

    def _lower_ordered_insts(self, ordered):
        _split_multi_waits(ordered)
        return super()._lower_ordered_insts(ordered)

    def _drain_and_barrier(self, tick_clock, wait_clock):
        gc = tick_clock.global_clock
        for proc in range(len(gc)):
            if gc[proc] <= 0:
                continue
            cur = VectorClock([0 if i == proc else gc[i] for i in range(len(gc))])
            nop = self.nc.sync.nop()
            wait_clock.add_sem_waits(
                nop.ins, ScopedClock({None: gc}), ScopedClock({None: cur})
            )
        drain_inst = self.nc.sync.drain()
        wait_clock.add_sem_waits(
            drain_inst.ins, ScopedClock({None: gc}), ScopedClock({None: gc.copy()})
        )
        self.nc.all_engine_barrier()
        assert self.sems is not None
        popped = self.nc._tile_sem_poison_stack.pop()
        assert popped is self._sem_poison
        self.nc.clear_and_free_semaphores(list(self.sems.allocated().values()))
        self.nc.all_engine_barrier()


def build_nc(reps=1):
    nc = bass.Bass()
    xT = nc.declare_dram_parameter("xT", [D, S], F32R, isOutput=False)
    wqT = nc.declare_dram_parameter("wqT", [D, E], F32R, isOutput=False)
    wkT = nc.declare_dram_parameter("wkT", [D, E], F32R, isOutput=False)
    wvT = nc.declare_dram_parameter("wvT", [D, E], F32R, isOutput=False)
    woT = nc.declare_dram_parameter("woT", [E, D], BF16, isOutput=False)
    ident_d = nc.declare_dram_parameter("ident", [128, 128], BF16, isOutput=False)
    out = nc.declare_dram_parameter("out_partial", [S, D], BF16, isOutput=True)

    with _TileContext(nc) as tc, ExitStack() as outer:
      for _rep in range(reps):
        ctx = outer.enter_context(ExitStack())
        # ---- persistent tiles ----
        act_pool = ctx.enter_context(tc.tile_pool(name="acts", bufs=1))
        qT_sb = [act_pool.tile([128, S], F32R, tag=f"qT{m}", name=f"qT{m}") for m in range(2)]
        kT_sb = [act_pool.tile([128, S], F32R, tag=f"kT{m}", name=f"kT{m}") for m in range(2)]
        v_sb = act_pool.tile([128, SB, HG, HD + 1], BF16, tag="v")
        wo_sb = act_pool.tile([128, 2, D], BF16, tag="wo")
        ident = act_pool.tile([128, 128], BF16, tag="id")
        attn_pair = [act_pool.tile([128, S], BF16, tag=f"ap{m}", name=f"ap{m}") for m in range(2)]

        # deferred-projection psum bank (right stack, below xqk so xqk can
        # close first); also reused for the overlapped output projection
        cdef = ExitStack()
        dpool = cdef.enter_context(
            tc.tile_pool(name="dp", bufs=1, space="PSUM", side="right")
        )

        # ---- x + weights (freed after the last deferred projection) ----
        c1x = ctx.enter_context(ExitStack())
        xqk_pool = c1x.enter_context(tc.tile_pool(name="xqk", bufs=1, side="right"))
        x_sb = xqk_pool.tile([128, KT, S], F32R, tag="x")
        wq_sb = xqk_pool.tile([128, KT, E], F32R, tag="wq")
        wk_sb = xqk_pool.tile([128, KT, E], F32R, tag="wk")
        cwv = ExitStack()
        wv_pool = cwv.enter_context(
            tc.tile_pool(name="wvp", bufs=1, side="right")
        )
        wv_sb = wv_pool.tile([128, KT, E], F32R, tag="wv")

        nc.vector.memset(v_sb[:, :, :, HD], 1.0)

        # DMA order (the model serializes the shared DMA device in issue
        # order): wq, wk, wv (so v projections can run in the pre-attention
        # PE window), then all of x, wo, ident.
        def dma_x(nb):
            for k in range(KT):
                eng = nc.sync if k % 2 == 0 else nc.gpsimd
                eng.dma_start(
                    x_sb[:, k, nb * 512:(nb + 1) * 512],
                    xT[k * 128:(k + 1) * 128, nb * 512:(nb + 1) * 512],
                )

        for k in range(KT):
            eng = nc.sync if k % 2 == 0 else nc.gpsimd
            eng.dma_start(wq_sb[:, k, :], wqT[k * 128:(k + 1) * 128, :])
        dma_x(0)
        for k in range(KT):
            eng = nc.sync if k % 2 == 0 else nc.gpsimd
            eng.dma_start(wk_sb[:, k, :], wkT[k * 128:(k + 1) * 128, :])
        for k in range(KT):
            eng = nc.sync if k % 2 == 0 else nc.gpsimd
            eng.dma_start(wv_sb[:, k, :], wvT[k * 128:(k + 1) * 128, :])
        for nb in range(1, 4):
            dma_x(nb)
        for m in range(2):
            eng = nc.sync if m == 0 else nc.gpsimd
            eng.dma_start(wo_sb[:, m, :], woT[m * 128:(m + 1) * 128, :])
        nc.sync.dma_start(ident[:], ident_d[:, :])

        def proj_qk(dst, w_sb, mcol, nb, ps):
            """dst[:, nb*512:+512] = (w column block mcol)^T x, via psum ps."""
            for k in range(KT):
                nc.tensor.matmul(
                    ps[:],
                    w_sb[:, k, mcol * 128:(mcol + 1) * 128],
                    x_sb[:, k, nb * 512:(nb + 1) * 512],
                    start=(k == 0),
                    stop=(k == KT - 1),
                )

        copy_flip = [0]
        copy_mode = ["startup"]  # pre-attention: ACT is idle

        def drain_copy(dst_ap, src_ap):
            # GPSIMD cannot access PSUM, so psum-draining copies go to DVE,
            # plus ACT while it is still idle (before the first exp)
            copy_flip[0] += 1
            if copy_mode[0] == "startup" and copy_flip[0] % 2 == 1:
                nc.scalar.copy(dst_ap, src_ap)
            else:
                nc.vector.tensor_copy(dst_ap, src_ap)

        # ---- phase 1a: qT0 half 0 + kT0 (gates the first attention group) --
        with ExitStack() as c1a:
            pp = c1a.enter_context(tc.tile_pool(name="pp", bufs=3, space="PSUM"))
            for nb in range(2):
                ps = pp.tile([128, 512], F32, tag="pp")
                proj_qk(qT_sb[0], wq_sb, 0, nb, ps)
                drain_copy(qT_sb[0][:, nb * 512:(nb + 1) * 512], ps[:])
            for nb in range(2):
                ps = pp.tile([128, 512], F32, tag="pp")
                proj_qk(kT_sb[0], wk_sb, 0, nb, ps)
                drain_copy(kT_sb[0][:, nb * 512:(nb + 1) * 512], ps[:])

        # ---- deferred projections.  PSUM allows only ONE accumulation
        # group per 2KB bank, so each unit owns a full bank; two tags
        # alternate banks so consecutive units overlap without parking in
        # PE's wait queue.  Units stream as low-priority hooks.
        dflip = [0]

        def dnext():
            dflip[0] += 1
            return dpool.tile(
                [128, 512], F32, tag="dA" if dflip[0] % 2 else "dB",
                name=f"dt{dflip[0]}",
            )

        def def_v(sb):
            """v projection for one 128-row s-block."""
            ps = dnext()[:, 0:256]
            for k in range(KT):
                nc.tensor.matmul(
                    ps,
                    x_sb[:, k, sb * 128:(sb + 1) * 128],
                    wv_sb[:, k, :],
                    start=(k == 0),
                    stop=(k == KT - 1),
                )
            drain_copy(
                v_sb[:, sb, :, 0:HD],
                ps.rearrange("p (h e) -> p h e", h=HG),
            )

        def def_qk(dst, w_sb, mcol, nb):
            """one 512-wide q/k output block."""
            ps = dnext()[:]
            proj_qk(dst, w_sb, mcol, nb, ps)
            drain_copy(dst[:, nb * 512:(nb + 1) * 512], ps)

        # v s-blocks 0-7 fit the pre-attention PE window (wv + x halves 0-1)
        for sb in range(8):
            def_v(sb)

        def qk_unit(dst, w_sb, mcol, nb):
            return lambda: def_qk(dst, w_sb, mcol, nb)

        # group order is (m0,qh0),(m0,qh1),(m1,qh0),(m1,qh1): group 1 reuses
        # kT0, so every deferred projection has at least a full group of
        # deadline slack.
        def_units = {
            0: (
                [qk_unit(kT_sb[0], wk_sb, 0, 2),
                 qk_unit(qT_sb[0], wq_sb, 0, 2)]
                + [(lambda s: (lambda: def_v(s)))(s) for s in (8, 9, 10)]
                + [qk_unit(kT_sb[0], wk_sb, 0, 3),
                   qk_unit(qT_sb[0], wq_sb, 0, 3)]
                + [(lambda s: (lambda: def_v(s)))(s) for s in (11, 12, 13, 14, 15)]
            ),
            1: [
                qk_unit(kT_sb[1], wk_sb, 1, 0),
                qk_unit(kT_sb[1], wk_sb, 1, 1),
                qk_unit(qT_sb[1], wq_sb, 1, 0),
                qk_unit(qT_sb[1], wq_sb, 1, 1),
                qk_unit(kT_sb[1], wk_sb, 1, 2),
                qk_unit(kT_sb[1], wk_sb, 1, 3),
            ],
            2: [
                qk_unit(qT_sb[1], wq_sb, 1, 2),
                qk_unit(qT_sb[1], wq_sb, 1, 3),
            ],
        }

        # ---- phase 2: attention groups ----
        # Phase A per group: scores + exp, retaining all 32 p tiles.
        # Phase B (hooked under the NEXT group's phase A): per (head,
        # qs-block) a 16-matmul accumulation chain through a single psum
        # bank (ones column gives the denominator in row 64), then
        # reciprocal + scale (bf16) + tensor-engine transpose back to the
        # [e, qs] head-pair tile.
        ost_pool = ctx.enter_context(tc.tile_pool(name="ost", bufs=4))
        grp = ctx.enter_context(ExitStack())
        sc_pool = grp.enter_context(tc.tile_pool(name="sc", bufs=1, space="PSUM"))
        pv_pool = grp.enter_context(tc.tile_pool(name="pv", bufs=1, space="PSUM"))
        p_pool = grp.enter_context(tc.tile_pool(name="pexp", bufs=27))
        st_pool = grp.enter_context(tc.tile_pool(name="stg", bufs=6))
        rd_pool = grp.enter_context(tc.tile_pool(name="rd", bufs=8))
        cast_flip = [0]

        def tail_cast(dst, src, use_act):
            cast_flip[0] += 1
            if use_act and cast_flip[0] % 2 == 0:
                nc.scalar.copy(dst, src)
            else:
                nc.vector.tensor_copy(dst, src)

        def tail_proj(sb, use_act=True):
            """output projection of one s-block via the deferred banks
            (DVE casts while ACT still runs exps; gpsimd cannot read
            psum)."""
            for nb in range(2):
                ps = dnext()[:]
                for mm in range(2):
                    nc.tensor.matmul(
                        ps,
                        attn_pair[mm][:, sb * 128:(sb + 1) * 128],
                        wo_sb[:, mm, nb * 512:(nb + 1) * 512],
                        start=(mm == 0),
                        stop=(mm == 1),
                    )
                st_o = ost_pool.tile([128, 512], BF16, tag="ost")
                tail_cast(st_o[:], ps, use_act)
                nc.sync.dma_start(
                    out[sb * 128:(sb + 1) * 128, nb * 512:(nb + 1) * 512],
                    st_o[:],
                )

        part_pool = grp.enter_context(tc.tile_pool(name="part", bufs=27))
        car_n = [0]

        def pv_car(tag):
            def alloc():
                car_n[0] += 1
                return pv_pool.tile(
                    [128, HD + 1], F32, tag=tag, name=f"car{car_n[0]}"
                )[:]
            return alloc

        mid_cars = [pv_car("pvA"), pv_car("pvB")]

        def chain(car, pts, m, r, qsb, k0):
            """8-matmul accumulation over ksb k0..k0+7 into a psum carrier."""
            for kk in range(8):
                ksb = k0 + kk
                nc.tensor.matmul(
                    car,
                    pts[r][ksb][:, qsb * 128:(qsb + 1) * 128],
                    v_sb[:, ksb, 2 * m + r, :],
                    start=(kk == 0),
                    stop=(kk == 7),
                )

        def make_half1(m, pts, parts, carriers):
            """First-half chains (ksb 0-7), partials parked in SBUF; hooked
            under the same group's second-half scores."""
            items = []

            def one(qsb, r):
                def run():
                    car = carriers[(qsb * 2 + r) % len(carriers)]()
                    chain(car, pts, m, r, qsb, 0)
                    pt = part_pool.tile([128, HD + 1], F32, tag="pt",
                                        name=f"pt{qsb}_{r}")
                    nc.vector.tensor_copy(pt[:], car)
                    parts[qsb * 2 + r] = pt
                return run

            for qsb in range(8):
                for r in range(2):
                    items.append(one(qsb, r))
            return items

        def make_half2(gi, m, qh, pts, parts, carriers, trt, n_trs, tail,
                      proj_base):
            """Second-half chains + partial add + normalize + transpose
            (+ output projection); hooked under the next group, or emitted
            directly as the tail for the last group."""
            items = []
            sts = {}

            def slot(qsb, r):
                def run():
                    car = carriers[(qsb * 2 + r) % len(carriers)]()
                    chain(car, pts, m, r, qsb, 8)
                    pt = parts[qsb * 2 + r]
                    nc.vector.tensor_add(pt[:], pt[:], car)
                    rden = rd_pool.tile([128, 1], F32, tag="rd")
                    nc.vector.reciprocal(rden[:], pt[:, 64:65])
                    if qsb * 2 + r in sts:
                        st = sts[qsb * 2 + r]
                    else:
                        st = st_pool.tile([128, 128], BF16, tag="st")
                        sts[qsb * 2] = st
                        sts[qsb * 2 + 1] = st
                    if tail and r == 0:
                        nc.scalar.mul(
                            st[:, r * 64:(r + 1) * 64], pt[:, 0:HD], rden[:]
                        )
                    else:
                        nc.vector.tensor_scalar_mul(
                            st[:, r * 64:(r + 1) * 64], pt[:, 0:HD], rden[:]
                        )
                return run

            def tr_item(qsb):
                def run():
                    s0 = (qsb % n_trs) * 64
                    tr_ap = trt()[:, s0:s0 + 64].bitcast(BF16)
                    nc.tensor.matmul(
                        tr_ap, sts[qsb * 2][:], ident[:], is_transpose=True
                    )
                    attn_dst = attn_pair[m][:, qh * 1024 + qsb * 128:
                                            qh * 1024 + (qsb + 1) * 128]
                    nc.vector.tensor_copy(attn_dst, tr_ap)
                    if proj_base is not None:
                        tail_proj(proj_base + qsb, use_act=tail)
                return run

            items.append(slot(0, 0))
            items.append(slot(0, 1))
            for qsb in range(1, 8):
                items.append(slot(qsb, 0))
                items.append(tr_item(qsb - 1))
                items.append(slot(qsb, 1))
            items.append(tr_item(7))
            return items

        def lowprio(fn):
            orig_prio = tc.cur_priority
            tc.cur_priority = orig_prio + 500000
            try:
                fn()
            finally:
                tc.cur_priority = orig_prio

        group_list = [(0, 0), (0, 1), (1, 0), (1, 1)]  # (m, qh)
        copy_mode[0] = "era"
        prevB = []
        for gi, (m, qh) in enumerate(group_list):
            # hook streams: previous group's second half (normal priority —
            # it recycles the p tiles the exps need) and deferred
            # projections (low priority)
            ghA = list(prevB)
            ghD = list(def_units.get(gi, []))
            pts = [[None] * SB, [None] * SB]
            parts = [None] * 16
            ghB = []   # own first-half chains, emitted under ksb 8-15
            for ksb in range(SB):
                for r in range(2):
                    sc = sc_pool.tile([128, 1024], F32, tag=f"sc{r}")
                    for qq in range(2):
                        nc.tensor.matmul(
                            sc[:, qq * 512:(qq + 1) * 512],
                            kT_sb[m][64 * r:64 * r + 64,
                                     ksb * 128:(ksb + 1) * 128],
                            qT_sb[m][64 * r:64 * r + 64,
                                     qh * 1024 + qq * 512:
                                     qh * 1024 + (qq + 1) * 512],
                            start=True,
                            stop=True,
                        )
                    p = p_pool.tile([128, 1024], BF16, tag="p", name=f"p{r}")
                    nc.scalar.activation(p[:], sc[:], EXP)
                    pts[r][ksb] = p
                if ksb == 7:
                    ghB = make_half1(m, pts, parts, mid_cars)
                for _ in range(4):
                    if ghA:
                        ghA.pop(0)()
                for _ in range(3):
                    if ghB:
                        ghB.pop(0)()
                if ghD:
                    lowprio(ghD.pop(0))
            for it in ghA + ghB:
                it()
            for it in ghD:
                lowprio(it)
            if gi == 0:
                cwv.close()    # wv SBUF free (v fully projected)
            elif gi == 2:
                c1x.close()    # x / wq / wk SBUF free
            if gi < 3:
                trm_n = [0]

                def trm():
                    trm_n[0] += 1
                    return pv_pool.tile([128, HD + 1], F32, tag="pvA",
                                        name=f"trm{gi}_{trm_n[0]}")
                prevB = make_half2(
                    gi, m, qh, pts, parts, mid_cars,
                    trt=trm, n_trs=1, tail=False,
                    proj_base=(0 if gi == 2 else None),
                )
            else:
                # last group: the second half is the tail.  Four carriers
                # (pv banks + freed sc0 banks), transposes in freed sc1.
                scc = sc_pool.tile([128, 1024], F32, tag="sc0", name="otc")
                tr3 = sc_pool.tile([128, 1024], F32, tag="sc1", name="tr3")
                tailB = make_half2(
                    gi, m, qh, pts, parts,
                    carriers=[
                        pv_car("pvA"), pv_car("pvB"),
                        lambda: scc[:, 0:HD + 1],
                        lambda: scc[:, 512:512 + HD + 1],
                    ],
                    trt=lambda: tr3, n_trs=8, tail=True,
                    proj_base=8,
                )
                for it in tailB:
                    it()
        grp.close()
        cdef.close()
        ctx.close()
    return nc


_NC_CACHE = None


def _get_nc():
    global _NC_CACHE
    if _NC_CACHE is None:
        _NC_CACHE = build_nc()
    return _NC_CACHE


_EXEC_CACHE = None


def _get_executor():
    """Build + jit the SPMD executable once; reuse across kernel() calls.

    Mirrors concourse.bass2jax.run_bass_via_pjrt, which re-jits on every
    call (full retrace + executable reload); caching shaves seconds/call."""
    global _EXEC_CACHE
    if _EXEC_CACHE is not None:
        return _EXEC_CACHE
    import jax
    from jax.sharding import Mesh, PartitionSpec
    from jax.experimental.shard_map import shard_map
    from concourse import bass2jax as b2j

    nc = _get_nc()
    b2j.install_neuronx_cc_hook()
    assert nc.dbg_addr is None
    partition_name = (
        nc.partition_id_tensor.name if nc.partition_id_tensor is not None else None
    )

    in_names, out_names, out_avals = [], [], []
    for alloc in nc.m.functions[0].allocations:
        if not isinstance(alloc, mybir.MemoryLocationSet):
            continue
        name = alloc.memorylocations[0].name
        if alloc.kind == "ExternalInput":
            if name != partition_name:
                in_names.append(name)
        elif alloc.kind == "ExternalOutput":
            out_names.append(name)
            out_avals.append(
                jax.core.ShapedArray(
                    tuple(alloc.tensor_shape), mybir.dt.np(alloc.dtype)
                )
            )
    n_params = len(in_names)
    n_outs = len(out_avals)
    all_names = in_names + out_names
    if partition_name is not None:
        all_names = all_names + [partition_name]

    def _body(*args):
        operands = list(args)
        if partition_name is not None:
            operands.append(b2j.partition_id_tensor())
        outs = b2j._bass_exec_p.bind(
            *operands,
            out_avals=tuple(out_avals),
            in_names=tuple(all_names),
            out_names=tuple(out_names),
            lowering_input_output_aliases=(),
            sim_require_finite=True,
            sim_require_nnan=True,
            nc=nc,
        )
        return tuple(outs)

    devices = jax.devices()[:NCORES]
    mesh = Mesh(np.asarray(devices), ("core",))
    donate = tuple(range(n_params, n_params + n_outs))
    sharded = jax.jit(
        shard_map(
            _body,
            mesh=mesh,
            in_specs=(PartitionSpec("core"),) * (n_params + n_outs),
            out_specs=(PartitionSpec("core"),) * n_outs,
            check_rep=False,
        ),
        donate_argnums=donate,
        keep_unused=True,
    )
    import jax.numpy as jnp

    zero_shardings = [
        jax.sharding.NamedSharding(mesh, PartitionSpec("core"))
    ] * n_outs

    @jax.jit
    def _make_zeros():
        return tuple(
            jax.lax.with_sharding_constraint(
                jnp.zeros((NCORES * a.shape[0], *a.shape[1:]), a.dtype), sh
            )
            for a, sh in zip(out_avals, zero_shardings)
        )

    _EXEC_CACHE = {
        "sharded": sharded,
        "make_zeros": _make_zeros,
        "in_names": in_names,
        "out_names": out_names,
        "out_avals": out_avals,
    }
    return _EXEC_CACHE


def _run_spmd(in_maps):
    ex = _get_executor()
    concat_in = [
        np.concatenate([np.asarray(m[name]) for m in in_maps], axis=0)
        for name in ex["in_names"]
    ]
    concat_zeros = ex["make_zeros"]()
    out_arrs = ex["sharded"](*concat_in, *concat_zeros)
    results = []
    for c in range(NCORES):
        results.append({
            name: np.asarray(out_arrs[i]).reshape(
                NCORES, *ex["out_avals"][i].shape
            )[c]
            for i, name in enumerate(ex["out_names"])
        })
    return results


def _shard_inputs(x, Wq, Wk, Wv, Wo):
    import ml_dtypes

    bf16 = ml_dtypes.bfloat16
    scale = np.float32(1.0 / np.sqrt(HD))
    ident = np.eye(128, dtype=bf16)
    in_maps = []
    xT_b = [np.ascontiguousarray(x[b].T) for b in range(B)]
    for c in range(NCORES):
        b, g = divmod(c, GROUPS)
        sl = slice(g * E, (g + 1) * E)
        in_maps.append({
            "ident": ident,
            "xT": xT_b[b],
            "wqT": np.ascontiguousarray(Wq[sl, :].T * scale),
            "wkT": np.ascontiguousarray(Wk[sl, :].T),
            "wvT": np.ascontiguousarray(Wv[sl, :].T),
            "woT": np.ascontiguousarray(Wo[:, sl].T).astype(bf16),
        })
    return in_maps


_FAST_PATH_OK = True


def kernel(x, Wq, Wk, Wv, Wo, bo):
    global _FAST_PATH_OK
    x = np.asarray(x, dtype=np.float32)
    in_maps = _shard_inputs(
        x,
        np.asarray(Wq, dtype=np.float32),
        np.asarray(Wk, dtype=np.float32),
        np.asarray(Wv, dtype=np.float32),
        np.asarray(Wo, dtype=np.float32),
    )
    results = None
    if _FAST_PATH_OK:
        try:
            results = _run_spmd(in_maps)
        except Exception:
            _FAST_PATH_OK = False
    if results is None:
        # portable fallback: stock SPMD runner (handles native-device
        # environments and anything the cached-PJRT fast path can't)
        results = run_bass_kernel_spmd(
            _get_nc(), in_maps, list(range(NCORES))
        ).results
    bo = np.asarray(bo, dtype=np.float32)
    out = np.empty((B, S, D), dtype=np.float32)
    for b in range(B):
        acc = np.zeros((S, D), dtype=np.float64)
        for g in range(GROUPS):
            acc += np.asarray(results[b * GROUPS + g]["out_partial"],
                              dtype=np.float64)
        out[b] = (acc + bo.astype(np.float64)).astype(np.float32)
    return out
